# revision 30
# baseline (speedup 1.0000x reference)
"""BinaryLinear kernel for Trainium2 (8 NeuronCores, SPMD).

Computes  out = sign(x) @ sign(W)^T * alpha  for
x: [8192, 2048] f32, W: [2048, 2048] f32, alpha: [1] f32.

Strategy: data-parallel over tokens (8 shards of 1024). Every core
reads W^T slices {0,1,2,3} as f32 locally (identical on all cores, so
the program stays SPMD-uniform) plus its OWN 256-col slice, which it
signs to fp8 and contributes to an HBM AllGather; slots {4..7} of the
AllGather output supply the remaining columns. Per-core HBM traffic
~21 MB vs 32 MB for full W replication.

Numerics: x tiles are signed to +-0.5 in one DVE op ((x>0) - 0.5) or
+-1 via ACT sign (late tiles, to unblock the DVE drain queue); W to
+-1 via ACT sign. fp8(E4M3) holds all exactly, PSUM sums are exact,
and each drain scales by 2*alpha or alpha per the tile's encoding.
Output is f16 (integers <= 2048 exact), converted to f32 on host.

DMA: three rings (scalar/Activation, sync/SP, gpsimd/SWDGE) balanced
by bytes, all major transfers with 4-8 KB per-partition runs. W01 is
split across scalar+sync so it lands early (it gates the first
matmul); w23 rides the gpsimd ring; the AllGather slot loads land in a
slot-major SBUF tile (contiguous 4 KB runs) consumed by FD-256
dual-accumulation-group DoubleRow units.
"""

import numpy as np

import concourse.bass as bass
import concourse.tile as tile
from concourse import bacc, mybir
from concourse.bass_utils import run_bass_kernel_spmd

N_CORES = 8
NTOK = 8192
INF = 2048
OUTF = 2048
TPC = NTOK // N_CORES  # tokens per core (1024)
P = 128
KT = INF // P  # 16 contraction tiles
MT = TPC // P  # 8 token tiles per core
SL = OUTF // N_CORES  # 256 out_features per W slice
FD = 512  # PSUM bank free dim

F32 = mybir.dt.float32
F16 = mybir.dt.float16
FP8 = mybir.dt.float8e4

# x tiles signed on ACT (+-1) instead of DVE (+-0.5). Empty for now: all
# x tiles go through the one-op DVE sign.
ACT_X_TILES = ()

# Static PE emission order over fine-grained (slice, m) units. Local
# slices 0-3 stream from f32 one slice at a time while x streams one
# m-tile at a time, so local units run in anti-diagonal (s+m) order to
# match joint arrival. AllGather slices 4-7 interleave from the middle
# (the AG completes mid-flight) with their m7 units last. Each unit is
# one FD-256 DoubleRow accumulation group in its own PSUM bank; 8
# consecutive units share one 4KB-run out DMA.
UNIT_ORDER = (
    [(0, 0), (1, 0), (0, 1), (1, 1), (0, 2), (1, 2), (0, 3), (1, 3)]
    + [(2, 0), (3, 0), (2, 1), (3, 1), (0, 4), (1, 4), (2, 2), (3, 2)]
    + [(0, 5), (1, 5), (2, 3), (3, 3), (0, 6), (1, 6), (2, 4), (3, 4)]
    + [(0, 7), (1, 7), (2, 5), (3, 5), (2, 6), (3, 6), (2, 7), (3, 7)]
    + [(s, m) for m in range(MT) for s in range(4, 8)]
)
assert len(UNIT_ORDER) == 64 and len(set(UNIT_ORDER)) == 64

_compiled = None
LAST_RESULT = None  # BassKernelResults of the most recent run (for profiling)


def _build():
    nc = bacc.Bacc(
        "TRN2",
        target_bir_lowering=False,
        debug=False,
        num_devices=N_CORES,
    )
    xt = nc.dram_tensor("xt", [MT * P * KT * P], F32, kind="ExternalInput").ap()
    # local W slices 0..3, each 2 chunks of [128, 8, 256] (8KB runs)
    wloc = nc.dram_tensor("wloc", [4 * P * KT * SL], F32, kind="ExternalInput").ap()
    # my slice, 2 chunks [128,8,256]
    wsl = nc.dram_tensor("wsl", [P * KT * SL], F32, kind="ExternalInput").ap()
    al = nc.dram_tensor("alpha", [P, 2], F32, kind="ExternalInput").ap()
    wsg_in = nc.dram_tensor("wsg_in", [P * KT * SL], FP8, kind="Internal")
    wsg_out = nc.dram_tensor(
        "wsg_out", [N_CORES * P * KT * SL], FP8, kind="Internal", addr_space="Shared"
    )
    out = nc.dram_tensor(
        "out", [8, P, 8 * SL], F16, kind="ExternalOutput"
    ).ap()

    with tile.TileContext(nc) as tc:
        with (
            tc.tile_pool(name="res", bufs=1) as res,
            tc.tile_pool(name="wload", bufs=4) as wload,
            tc.tile_pool(name="wsload", bufs=2) as wsload,
            tc.tile_pool(name="xload", bufs=4) as xload,
            tc.tile_pool(name="psum", bufs=8, space="PSUM") as ppool,
            tc.tile_pool(name="outp", bufs=3) as outp,
        ):
            bx = res.tile([P, KT, TPC], FP8)  # x signs, 16 KB/part
            bwl = res.tile([P, KT, 2 * FD], FP8)  # slices 0-3, 16 KB/part
            # AG slots 4-7, slot-major so loads are contiguous 4KB runs
            bwr = res.tile([P, 4, KT, SL], FP8)  # 16 KB/part
            bsl = res.tile([P, KT, SL], FP8)  # my slice fp8, 4 KB/part
            alpha_t = res.tile([P, 2], F32)  # [2*alpha, alpha] from host

            nc.scalar.dma_start(alpha_t[:], al)

            # emission helpers ------------------------------------------------
            def w_chunk(ring, s, half):
                # half-slice chunk: k-tiles [half*8, half*8+8) of slice s
                wf = wload.tile([P, 8, SL], F32, name="wf", tag="wf")
                base = (2 * s + half) * P * 8 * SL
                src = wloc[base : base + P * 8 * SL].rearrange("(p f) -> p f", p=P)
                ring.dma_start(wf[:].rearrange("p a b -> p (a b)"), src)
                nc.scalar.sign(
                    bwl[:, half * 8 : (half + 1) * 8, s * SL : (s + 1) * SL], wf[:]
                )

            def x_chunk(ring, m):
                xf = xload.tile([P, KT, P], F32, name="xf", tag="xf")
                src = xt[m * P * KT * P : (m + 1) * P * KT * P].rearrange(
                    "(p f) -> p f", p=P
                )
                ring.dma_start(xf[:].rearrange("p a b -> p (a b)"), src)
                if m in ACT_X_TILES:
                    nc.scalar.sign(bx[:, :, m * P : (m + 1) * P], xf[:])
                else:
                    nc.vector.tensor_scalar(
                        bx[:, :, m * P : (m + 1) * P], xf[:], 0.0, 0.5,
                        op0=mybir.AluOpType.is_gt, op1=mybir.AluOpType.subtract,
                    )

            def wsl_chunk(i):
                # my slice streams on the gpsimd ring; signed to +-0.5 on the
                # gpsimd engine so ACT/DVE stay free (AG drains scale by 4a).
                wsf = wsload.tile([P, 8, SL], F32, name="wsf", tag="wsf")
                src = wsl[i * P * 8 * SL : (i + 1) * P * 8 * SL].rearrange(
                    "(p f) -> p f", p=P
                )
                nc.gpsimd.dma_start(wsf[:].rearrange("p a b -> p (a b)"), src)
                nc.gpsimd.tensor_scalar(
                    bsl[:, i * 8 : (i + 1) * 8, :], wsf[:], 0.0, 0.5,
                    op0=mybir.AluOpType.is_gt, op1=mybir.AluOpType.subtract,
                )

            # ring schedules --------------------------------------------------
            # scalar: alpha x0 w0a w1a x2 w2a x4 w3a x6
            # sync:   w0b x1 w1b x3 w2b x5 w3b x7
            # gpsimd: wsl, bounce, (AG), agloads, outs — the whole AG chain
            #         plus output traffic rides the SWDGE ring.
            wsl_chunk(0)
            wsl_chunk(1)
            nc.gpsimd.dma_start(
                wsg_in.ap().rearrange("(p f) -> p f", p=P),
                bsl[:].rearrange("p a b -> p (a b)"),
            )
            x_chunk(nc.scalar, 0)
            w_chunk(nc.sync, 0, 1)
            w_chunk(nc.scalar, 0, 0)
            x_chunk(nc.sync, 1)
            w_chunk(nc.scalar, 1, 0)
            w_chunk(nc.sync, 1, 1)
            x_chunk(nc.scalar, 2)
            x_chunk(nc.sync, 3)
            w_chunk(nc.scalar, 2, 0)
            w_chunk(nc.sync, 2, 1)
            x_chunk(nc.scalar, 4)
            x_chunk(nc.sync, 5)
            w_chunk(nc.scalar, 3, 0)
            w_chunk(nc.sync, 3, 1)
            x_chunk(nc.scalar, 6)
            x_chunk(nc.sync, 7)

            nc.gpsimd.collective_compute(
                "AllGather",
                mybir.AluOpType.bypass,
                replica_groups=[list(range(N_CORES))],
                ins=[wsg_in.ap()],
                outs=[wsg_out.ap()],
            )
            # AG slot loads (gpsimd ring tail, gated on the AG): slots 4..7
            SLB = P * KT * SL
            for s in range(4, 8):
                src = wsg_out.ap()[s * SLB : (s + 1) * SLB].rearrange(
                    "(p f) -> p f", p=P
                )
                nc.gpsimd.dma_start(
                    bwr[:, s - 4, :, :].rearrange("p a b -> p (a b)"), src
                )

            # -- PE units ----------------------------------------------------
            # one FD-256 DoubleRow accumulation group per unit, in its own
            # PSUM bank (allocated [P, FD] so banks are never shared).
            ob8 = None
            for ui, (s, m) in enumerate(UNIT_ORDER):
                ps = ppool.tile([P, FD], F32, name="ps", tag="ps")
                if s < 4:
                    rhs = lambda kc: bwl[:, 2 * kc : 2 * kc + 2, s * SL : (s + 1) * SL]
                else:
                    rhs = lambda kc: bwr[:, s - 4, 2 * kc : 2 * kc + 2, :]
                for kc in range(KT // 2):
                    nc.tensor.matmul(
                        ps[:, 0:SL],
                        bx[:, 2 * kc : 2 * kc + 2, m * P : (m + 1) * P],
                        rhs(kc),
                        start=(kc == 0),
                        stop=(kc == KT // 2 - 1),
                        perf_mode=mybir.MatmulPerfMode.DoubleRow,
                    )
                if ui % 8 == 0:
                    ob8 = outp.tile([P, 8, SL], F16, name="ob", tag="ob")
                # col0 = 2*alpha (x +-0.5, W +-1); col1 = 4*alpha (AG slices:
                # both operands +-0.5)
                acol = 1 if s >= 4 else 0
                nc.vector.tensor_scalar_mul(
                    ob8[:, ui % 8, :], ps[:, 0:SL], alpha_t[:, acol : acol + 1]
                )
                if ui % 8 == 7:
                    nc.gpsimd.dma_start(
                        out[ui // 8], ob8[:].rearrange("p a b -> p (a b)")
                    )

    nc.compile()
    return nc


def _pack_common(weight):
    WT4 = np.ascontiguousarray(weight.T).reshape(KT, P, OUTF)

    def slice_chunks(s):
        cols = slice(s * SL, (s + 1) * SL)
        return [
            WT4[i * 8 : (i + 1) * 8, :, cols].transpose(1, 0, 2).ravel()
            for i in range(2)
        ]

    wloc = np.ascontiguousarray(
        np.concatenate([c for s in range(4) for c in slice_chunks(s)])
    )
    wsls = [
        np.ascontiguousarray(np.concatenate(slice_chunks(c)))
        for c in range(N_CORES)
    ]
    return wloc, wsls


def _pack_x_shard(xs):
    xT4 = np.ascontiguousarray(xs.T).reshape(KT, P, TPC)
    return np.ascontiguousarray(
        np.concatenate(
            [xT4[:, :, m * P : (m + 1) * P].transpose(1, 0, 2).ravel() for m in range(MT)]
        )
    )


def kernel(x, weight, alpha):
    global _compiled, LAST_RESULT
    if _compiled is None:
        _compiled = _build()
    nc = _compiled

    x = np.asarray(x, dtype=np.float32)
    weight = np.asarray(weight, dtype=np.float32)
    alpha = np.asarray(alpha, dtype=np.float32)

    wloc, wsls = _pack_common(weight)
    a = float(alpha.reshape(-1)[0])
    alv = np.empty((P, 2), dtype=np.float32)
    alv[:, 0] = 2.0 * a
    alv[:, 1] = 4.0 * a
    in_maps = []
    for c in range(N_CORES):
        xs = _pack_x_shard(x[c * TPC : (c + 1) * TPC, :])
        in_maps.append({"xt": xs, "wloc": wloc, "wsl": wsls[c], "alpha": alv})

    LAST_RESULT = run_bass_kernel_spmd(nc, in_maps, list(range(N_CORES)))
    full = np.empty((NTOK, OUTF), dtype=np.float32)
    for c in range(N_CORES):
        o = LAST_RESULT.results[c]["out"].astype(np.float32)  # [8, P, 8*SL]
        o = o.reshape(8, P, 8, SL)
        for ui, (s, m) in enumerate(UNIT_ORDER):
            rows = slice(c * TPC + m * P, c * TPC + (m + 1) * P)
            cols = slice(s * SL, (s + 1) * SL)
            full[rows, cols] = o[ui // 8, :, ui % 8, :]
    return full
